# revision 63
# baseline (speedup 1.0000x reference)
"""Trainium2 Bass kernel for nn_CrossAttention (B=2, N=2048, C=1024, H=16, D=64).

Strategy: sequence-parallel SPMD over 8 NeuronCores. Core i owns 512 rows of
the flattened [B*N, C] token axis (cores 0-3 = batch 0, cores 4-7 = batch 1).

Key design points (v3):
  - all big inputs are cast to bf16 on the HOST, so every device load is a
    fast non-casting HWDGE DMA (the SWDGE casting path delivered the first
    tile only after ~27us and delayed the whole K projection)
  - x_s/x_t are transposed by the DMA X-bar (dma_start transpose=True)
    straight from DRAM -> no PE transposes, no drain copies
  - PE order: k proj -> fire K AllGather in 2 chunks -> v proj -> fire V
    AllGather in 2 chunks (tt-major) -> q proj -> attention -> fuse.
    Collectives are issued from the otherwise-empty gpsimd queue.
  - gathered k lands in persistent kTf SBUF tiles (loads wait only on the
    collective), gathered v lands via strided DMA directly in the
    [V_h | 1] stationary layout (ones column -> softmax row sums for free)
  - attention in head pairs: st [128,2,512] f32 PSUM double-buffered so
    S^T(ki+1) overlaps exp(ki); the two S^T matmuls of a pair run
    concurrently in different PE row-tiles (tile_position)
  - deep pt pool so the exp stream can run ~32 iterations ahead of P@V
    while the V gather is still in flight
  - per-pair normalize: row sums staged at partition 0, DRAM-bounce
    partition broadcast, reciprocal_approx_fast, one multiply
"""

import sys

if "/opt/trn_rl_repo" not in sys.path:
    sys.path.insert(0, "/opt/trn_rl_repo")

import numpy as np

B, N, C, H, D = 2, 2048, 1024, 16, 64
NCORES = 8
T = (B * N) // NCORES          # 512 tokens per core
P = 128
SCALE = D ** -0.5              # 0.125
GROUPS = [[0, 1, 2, 3], [4, 5, 6, 7]]

# kt processing order: tt-major so P@V only needs v chunk tt after the vAG
# chunk covering that tt has landed. kt identifies (r=group member, tt).
KT_ORDER = [r * 4 + tt for tt in range(4) for r in range(4)]

_CACHE = {}


def _build():
    import concourse.bass as bass
    import concourse.mybir as mybir
    import concourse.tile as tile
    from concourse import bacc
    from concourse.masks import make_identity

    f32 = mybir.dt.float32
    bf16 = mybir.dt.bfloat16
    f8 = mybir.dt.float8e4      # e4m3: wire format for the k/v gathers

    nc = bacc.Bacc("TRN2", num_devices=NCORES, debug=False, enable_asserts=False)

    x_t = nc.dram_tensor("x_t", [T, C], bf16, kind="ExternalInput").ap()
    x_s = nc.dram_tensor("x_s", [T, C], bf16, kind="ExternalInput").ap()
    w_q = nc.dram_tensor("W_q", [C, C], bf16, kind="ExternalInput").ap()
    w_kv = nc.dram_tensor("W_kv", [C, 2 * C], bf16, kind="ExternalInput").ap()
    w_f = nc.dram_tensor("W_fuse", [C, C], bf16, kind="ExternalInput").ap()
    b_f = nc.dram_tensor("b_fuse", [1, C], f32, kind="ExternalInput").ap()
    out = nc.dram_tensor("out", [T, C], bf16, kind="ExternalOutput").ap()

    with tile.TileContext(nc) as tc:
        import contextlib

        with contextlib.ExitStack() as stk:
            consts = stk.enter_context(tc.tile_pool(name="consts", bufs=1))
            dram = stk.enter_context(tc.tile_pool(name="dram", bufs=1, space="DRAM"))

            identity = consts.tile([P, P], bf16, name="identity")
            make_identity(nc, identity)

            # preload the Exp activation table (saves 1.3us at first exp)
            dact = consts.tile([1, 2], f32, name="dact")
            nc.vector.memset(dact, 0.0)
            dact2 = consts.tile([1, 2], f32, name="dact2")
            nc.scalar.activation(dact2, dact,
                                 mybir.ActivationFunctionType.Exp, scale=1.0)

            bias_b = consts.tile([P, C], f32, name="bias_b")
            qT = [consts.tile([P, T], bf16, name=f"qT{m}") for m in range(8)]
            aT = [consts.tile([P, T], bf16, name=f"aT{c}") for c in range(8)]
            wf_b = consts.tile([P, 8, C], bf16, name="wf_b")
            wf = [wf_b[:, c, :] for c in range(8)]
            kTf = [consts.tile([P, 4, T], bf16, name=f"kTf{m}") for m in range(8)]
            vp = [consts.tile([P, H, D + 1], bf16, name=f"vp{kt}")
                  for kt in range(16)]

            # DRAM bounce buffers for the collectives. NOTE: fp8 wire format
            # was tried for k and/or v (halves collective bytes, -74us) but
            # pushed rel err to 2.05e-2/2.64e-2, over the 2e-2 gate.
            # K goes in ONE gather (all m-tiles ready together, early); V in
            # two dim-half gathers (heads 0-7 then 8-15) matching the pair
            # consumption order of the attention loop.
            k_in1 = dram.tile([2 * P * T], bf16, name="k_in1")   # m0-1
            k_in1b = dram.tile([2 * P * T], bf16, name="k_in1b")  # m2-3
            k_in2 = dram.tile([4 * P * T], bf16, name="k_in2")   # m4-7
            k_out1 = dram.tile([8 * P * T], bf16, name="k_out1")
            k_out1b = dram.tile([8 * P * T], bf16, name="k_out1b")
            k_out2 = dram.tile([16 * P * T], bf16, name="k_out2")
            v_in_lo = dram.tile([4 * P * 512], bf16, name="v_in_lo")
            v_in_hi = dram.tile([4 * P * 512], bf16, name="v_in_hi")
            v_out_lo = dram.tile([16 * P * 512], bf16, name="v_out_lo")
            v_out_hi = dram.tile([16 * P * 512], bf16, name="v_out_hi")
            rdram = dram.tile([H * T], f32, name="rdram")

            _k1r = k_in1.rearrange("(m p t) -> m p t", m=2, p=P, t=T)
            _k1br = k_in1b.rearrange("(m p t) -> m p t", m=2, p=P, t=T)
            _k2r = k_in2.rearrange("(m p t) -> m p t", m=4, p=P, t=T)
            k_in_v = [_k1r[0], _k1r[1], _k1br[0], _k1br[1],
                      _k2r[0], _k2r[1], _k2r[2], _k2r[3]]
            v_in_lo_v = v_in_lo.rearrange("(q p c) -> q p c", q=4, p=P, c=512)
            v_in_hi_v = v_in_hi.rearrange("(q p c) -> q p c", q=4, p=P, c=512)
            _ko1 = k_out1.rearrange("(r m p t) -> m p r t", r=4, m=2, p=P, t=T)
            _ko1b = k_out1b.rearrange("(r m p t) -> m p r t", r=4, m=2, p=P, t=T)
            _ko2 = k_out2.rearrange("(r m p t) -> m p r t", r=4, m=4, p=P, t=T)
            k_out_v = [_ko1[0], _ko1[1], _ko1b[0], _ko1b[1],
                       _ko2[0], _ko2[1], _ko2[2], _ko2[3]]
            v_out_lo_v = v_out_lo.rearrange("(r q p c) -> r q p c",
                                            r=4, q=4, p=P, c=512)
            v_out_hi_v = v_out_hi.rearrange("(r q p c) -> r q p c",
                                            r=4, q=4, p=P, c=512)
            rdram_v = rdram.rearrange("(h t) -> h t", h=H)

            def cc_allgather(inb, outb):
                nc.gpsimd.collective_compute(
                    "AllGather", mybir.AluOpType.bypass, replica_groups=GROUPS,
                    ins=[inb[:].opt()], outs=[outb[:].opt()])

            # warm up the collective stream: the first CC op of a kernel pays
            # ~20-25us of startup; burn it on a 128B dummy while the k
            # projection is still running
            warm_in = dram.tile([128], bf16, name="warm_in")
            warm_out = dram.tile([512], bf16, name="warm_out")
            cc_allgather(warm_in, warm_out)

            # ---------------- phase A: projections ----------------
            # pak outlives phase A: wk/xsT/kl feed the m4-7 K projection
            # that is interleaved into attention pair 0
            pak = stk.enter_context(tc.tile_pool(name="pak", bufs=1))
            with tc.tile_pool(name="pa", bufs=1) as pa, \
                 tc.tile_pool(name="kp_ps", bufs=1, space="PSUM") as kp_ps, \
                 tc.tile_pool(name="tp_ps", bufs=2, space="PSUM") as tp_ps, \
                 tc.tile_pool(name="pp_ps", bufs=2, space="PSUM") as pp_ps:

                # batched loads: few big 3D-AP DMAs instead of many small
                # tiles — per-DMA latency (~3-6us each) was pacing phase A.
                # x on the sync queue, weights on the gpsimd queue.
                xs_b = pa.tile([P, 4, C], bf16, name="xs_b")
                xs_r = x_s.rearrange("(i p) c -> p i c", p=P)
                xt_b = pa.tile([P, 4, C], bf16, name="xt_b")
                xt_r = x_t.rearrange("(i p) c -> p i c", p=P)
                # first x_s row-tile alone so the transposes start ~6us sooner
                nc.sync.dma_start(out=xs_b[:, 0:1, :], in_=xs_r[:, 0:1, :])
                nc.sync.dma_start(out=xs_b[:, 1:4, :], in_=xs_r[:, 1:4, :])
                for h in range(2):
                    nc.sync.dma_start(out=xt_b[:, 2 * h:2 * h + 2, :],
                                      in_=xt_r[:, 2 * h:2 * h + 2, :])
                xs_nat = [xs_b[:, i, :] for i in range(4)]
                xt_nat = [xt_b[:, i, :] for i in range(4)]

                wk_b = pak.tile([P, 8, C], bf16, name="wk_b")
                wk_r = w_kv[:, 0:C].rearrange("(c p) k -> p c k", p=P)
                wv_b = pa.tile([P, 8, C], bf16, name="wv_b")
                wv_r = w_kv[:, C:2 * C].rearrange("(c p) k -> p c k", p=P)
                wq_b = pa.tile([P, 8, C], bf16, name="wq_b")
                wq_r = w_q.rearrange("(c p) k -> p c k", p=P)
                for h in range(2):
                    nc.gpsimd.dma_start(out=wk_b[:, 4 * h:4 * h + 4, :],
                                        in_=wk_r[:, 4 * h:4 * h + 4, :])
                for h in range(2):
                    nc.gpsimd.dma_start(out=wv_b[:, 4 * h:4 * h + 4, :],
                                        in_=wv_r[:, 4 * h:4 * h + 4, :])
                wk = [wk_b[:, c, :] for c in range(8)]
                wv = [wv_b[:, c, :] for c in range(8)]
                wq = [wq_b[:, c, :] for c in range(8)]

                def transpose_in(nat, dstT):
                    for i in range(4):
                        for c in range(8):
                            pst = tp_ps.tile([P, P], bf16, name="pst")
                            nc.tensor.transpose(
                                pst, nat[i][:, c * P:(c + 1) * P], identity)
                            nc.vector.tensor_copy(
                                out=dstT[c][:, i * P:(i + 1) * P], in_=pst)

                xsT = [pak.tile([P, T], bf16, name=f"xsT{c}") for c in range(8)]
                transpose_in(xs_nat, xsT)

                # k^T projection m0-3, gathered in two 2-m chunks (the first
                # covers pairs 0-1 and gates exp start); m4-7 is deferred to
                # a prefix of attention pair 0, where PE would otherwise idle
                for mg in range(2):
                    pss = [kp_ps.tile([P, T], f32, name="kps", tag=f"kps{j}")
                           for j in range(2)]
                    for c in range(8):
                        for j in range(2):
                            m = 2 * mg + j
                            nc.tensor.matmul(
                                pss[j], wk[c][:, m * P:(m + 1) * P], xsT[c],
                                start=(c == 0), stop=(c == 7))
                    for j in range(2):
                        kl = pak.tile([P, T], bf16, name="kl", bufs=4)
                        nc.vector.tensor_copy(out=kl, in_=pss[j])
                        nc.gpsimd.dma_start(out=k_in_v[2 * mg + j], in_=kl)
                    cc_allgather([k_in1, k_in1b][mg], [k_out1, k_out1b][mg])

                # wq rides the gpsimd queue behind the k_in writes; vp ones
                # columns prime on vector once the kl drains are queued
                for h in range(2):
                    nc.gpsimd.dma_start(out=wq_b[:, 4 * h:4 * h + 4, :],
                                        in_=wq_r[:, 4 * h:4 * h + 4, :])
                for kt in range(16):
                    nc.vector.memset(vp[kt], 1.0)

                # v projection (natural layout), nh-outer: the lo dim-half
                # (heads 0-7, consumed by pairs 0-3) completes and gathers
                # first; kAG2 slots between the two v gathers (its consumers
                # run last)
                for nh in range(2):
                    for tt in range(4):
                        ps = pp_ps.tile([P, 512], f32, name="proj_ps")
                        for c in range(8):
                            nc.tensor.matmul(
                                ps,
                                xsT[c][:, tt * P:(tt + 1) * P],
                                wv[c][:, nh * 512:(nh + 1) * 512],
                                start=(c == 0), stop=(c == 7))
                        vl = pa.tile([P, 512], bf16, name="vl", bufs=4)
                        nc.vector.tensor_copy(out=vl, in_=ps)
                        nc.gpsimd.dma_start(
                            out=[v_in_lo_v, v_in_hi_v][nh][tt], in_=vl)
                    if nh == 0:
                        cc_allgather(v_in_lo, v_out_lo)
                # kAG2 and vAG-hi are emitted from inside attention pair 0,
                # once the interleaved m4-7 projection lands

                # q^T projection
                xtT = [pa.tile([P, T], bf16, name=f"xtT{c}") for c in range(8)]
                transpose_in(xt_nat, xtT)
                for m in range(8):
                    ps = pp_ps.tile([P, T], f32, name="proj_ps")
                    for c in range(8):
                        nc.tensor.matmul(ps, wq[c][:, m * P:(m + 1) * P], xtT[c],
                                         start=(c == 0), stop=(c == 7))
                    nc.vector.tensor_copy(out=qT[m], in_=ps)

                # late loads: W_fuse + bias ride behind the collectives
                wf_r = w_f.rearrange("(c p) k -> p c k", p=P)
                for h in range(2):
                    nc.gpsimd.dma_start(out=wf_b[:, 4 * h:4 * h + 4, :],
                                        in_=wf_r[:, 4 * h:4 * h + 4, :])
                nc.gpsimd.dma_start(out=bias_b, in_=b_f.to_broadcast([P, C]))

            # gathered k m0-3 -> kTf (sync queue, dep kAG1/kAG1b); the rest on
            # the gpsimd queue in collective-completion order: vp-lo (vAG-lo),
            # kTf m4-7 (kAG2), vp-hi (vAG-hi)
            for m in range(4):
                nc.sync.dma_start(out=kTf[m], in_=k_out_v[m])
            for kt in KT_ORDER:
                r, ttv = kt // 4, kt % 4
                nc.gpsimd.dma_start(
                    out=vp[kt][:, 0:8, 0:D],
                    in_=v_out_lo_v[r, ttv].rearrange("p (h d) -> p h d", h=8))
            # (kTf m4-7 and vp-hi loads are emitted after attention pair 0 —
            # they must follow the kAG2/vAG-hi emission in program order for
            # dependency tracking, and must not sit on the gpsimd queue ahead
            # of those launches)

            # ---------------- phase B: attention ----------------
            with tc.tile_pool(name="ptp", bufs=30) as ptp, \
                 tc.tile_pool(name="sm", bufs=2) as sm:

                def emit_st(stp, hp, ki):
                    kt = KT_ORDER[ki]
                    r, tcol = kt // 4, (kt % 4) * P
                    st = stp.tile([P, 2, T], f32, name="st")
                    for sub in range(2):
                        nc.tensor.matmul(
                            st[:, sub, :],
                            kTf[hp][sub * D:(sub + 1) * D, r, tcol:tcol + P],
                            qT[hp][sub * D:(sub + 1) * D, :],
                            start=True, stop=True,
                            tile_position=(sub * D, 0))
                    return st

                def pair_body(hp, stp, otp, extra):
                    ot = [otp.tile([D + 1, T], f32, name="ot", tag=f"ot{i}")
                          for i in range(2)]
                    st = emit_st(stp, hp, 0)
                    for ki in range(16):
                        kt = KT_ORDER[ki]
                        pt = ptp.tile([P, 2, T], bf16, name="pt")
                        nc.scalar.activation(
                            pt[:], st[:],
                            mybir.ActivationFunctionType.Exp, scale=SCALE)
                        if ki < 15:
                            st = emit_st(stp, hp, ki + 1)
                        for i in range(2):
                            nc.tensor.matmul(
                                ot[i], vp[kt][:, 2 * hp + i, :], pt[:, i, :],
                                start=(ki == 0), stop=(ki == 15))
                        if extra is not None:
                            extra(2 * ki)
                            extra(2 * ki + 1)
                    # drain pair: O^T -> aT[hp]; row sums staged at partition
                    # 0, DRAM-bounced into a partition broadcast, then one
                    # fast reciprocal + multiply normalizes the pair
                    for i in range(2):
                        nc.vector.tensor_copy(
                            out=aT[hp][i * D:(i + 1) * D, :], in_=ot[i][0:D, :])
                        rs = sm.tile([1, T], f32, name="rs", tag=f"rs{i}")
                        nc.vector.tensor_copy(out=rs, in_=ot[i][D:D + 1, :])
                        nc.sync.dma_start(out=rdram_v[2 * hp + i], in_=rs)
                    rb = sm.tile([P, T], f32, name="rb")
                    for half in range(2):
                        bcast = bass.AP(
                            tensor=rdram.tensor,
                            offset=rdram.offset + (2 * hp + half) * T,
                            ap=[[0, D], [1, T]])
                        nc.gpsimd.dma_start(
                            out=rb[half * D:(half + 1) * D, :], in_=bcast)
                    rbi = sm.tile([P, T], f32, name="rbi")
                    nc.vector.reciprocal_approx_fast(out=rbi, in_=rb)
                    nc.vector.tensor_mul(out=aT[hp], in0=aT[hp], in1=rbi)

                # the deferred m4-7 K projection runs as a PREFIX of pair 0 —
                # this is exactly the window where PE otherwise idles waiting
                # for kAG1 to land; kAG2 + vAG-hi fire as soon as it drains
                with tc.tile_pool(name="st0_ps", bufs=2, space="PSUM") as st0, \
                     tc.tile_pool(name="ot0_ps", bufs=1, space="PSUM") as ot0, \
                     tc.tile_pool(name="kp2_ps", bufs=1, space="PSUM") as kp2:
                    for sub in range(2):
                        pss2 = [kp2.tile([P, T], f32, name="kps2",
                                         tag=f"kps2_{j}") for j in range(2)]
                        for c in range(8):
                            for j in range(2):
                                m = 4 + 2 * sub + j
                                nc.tensor.matmul(
                                    pss2[j], wk[c][:, m * P:(m + 1) * P],
                                    xsT[c], start=(c == 0), stop=(c == 7))
                        for j in range(2):
                            m = 4 + 2 * sub + j
                            kl = pak.tile([P, T], bf16, name="kl", bufs=4)
                            nc.vector.tensor_copy(out=kl, in_=pss2[j])
                            nc.gpsimd.dma_start(out=k_in_v[m], in_=kl)
                    cc_allgather(k_in2, k_out2)
                    cc_allgather(v_in_hi, v_out_hi)

                    pair_body(0, st0, ot0, None)

                # now that kAG2/vAG-hi exist in program order, load their
                # results (sync queue; gpsimd carries the launches)
                for m in range(4, 8):
                    nc.sync.dma_start(out=kTf[m], in_=k_out_v[m])
                for kt in KT_ORDER:
                    r, ttv = kt // 4, kt % 4
                    nc.sync.dma_start(
                        out=vp[kt][:, 8:16, 0:D],
                        in_=v_out_hi_v[r, ttv].rearrange(
                            "p (h d) -> p h d", h=8))

                with tc.tile_pool(name="st_ps", bufs=2, space="PSUM") as st_ps, \
                     tc.tile_pool(name="ot_ps", bufs=2, space="PSUM") as ot_ps:
                    for hp in range(1, 8):
                        pair_body(hp, st_ps, ot_ps, None)

            # ---------------- phase C: fuse projection ----------------
            with tc.tile_pool(name="fu", bufs=4) as fu, \
                 tc.tile_pool(name="fu_ps", bufs=4, space="PSUM") as fu_ps:
                for tt in range(4):
                    for nh in range(2):
                        ps = fu_ps.tile([P, 512], f32, name="fps")
                        for c in range(8):
                            nc.tensor.matmul(
                                ps, aT[c][:, tt * P:(tt + 1) * P],
                                wf[c][:, nh * 512:(nh + 1) * 512],
                                start=(c == 0), stop=(c == 7))
                        ob = fu.tile([P, 512], bf16, name="ob")
                        nc.vector.tensor_add(
                            out=ob, in0=ps, in1=bias_b[:, nh * 512:(nh + 1) * 512])
                        nc.sync.dma_start(
                            out=out[tt * P:(tt + 1) * P, nh * 512:(nh + 1) * 512],
                            in_=ob)

    nc.compile()
    return nc


def _get_nc():
    if "nc" not in _CACHE:
        _CACHE["nc"] = _build()
    return _CACHE["nc"]


def make_in_maps(inputs):
    """Shard + host-cast the full inputs into per-core input maps."""
    import ml_dtypes

    bf16 = ml_dtypes.bfloat16
    x_t = np.asarray(inputs["x_t"]).reshape(B * N, C).astype(bf16)
    x_s = np.asarray(inputs["x_s"]).reshape(B * N, C).astype(bf16)
    w_q = np.asarray(inputs["W_q"]).astype(bf16)
    w_kv = np.asarray(inputs["W_kv"]).astype(bf16)
    w_f = np.asarray(inputs["W_fuse"]).astype(bf16)
    b_f = np.asarray(inputs["b_fuse"]).astype(np.float32).reshape(1, C)

    in_maps = []
    for i in range(NCORES):
        in_maps.append({
            "x_t": x_t[i * T:(i + 1) * T],
            "x_s": x_s[i * T:(i + 1) * T],
            "W_q": w_q,
            "W_kv": w_kv,
            "W_fuse": w_f,
            "b_fuse": b_f,
        })
    return in_maps


def kernel(**inputs):
    nc = _get_nc()
    from concourse import bass_utils

    in_maps = make_in_maps(inputs)
    res = bass_utils.run_bass_kernel_spmd(nc, in_maps, core_ids=list(range(NCORES)))
    out = np.concatenate([res.results[i]["out"] for i in range(NCORES)], axis=0)
    return out.reshape(B, N, C).astype(np.float32)


if __name__ == "__main__":
    _build()
    print("build+compile OK")


# revision 65
# speedup vs baseline: 1.0217x; 1.0217x over previous
"""Trainium2 Bass kernel for nn_CrossAttention (B=2, N=2048, C=1024, H=16, D=64).

Strategy: sequence-parallel SPMD over 8 NeuronCores. Core i owns 512 rows of
the flattened [B*N, C] token axis (cores 0-3 = batch 0, cores 4-7 = batch 1).

Key design points (v3):
  - all big inputs are cast to bf16 on the HOST, so every device load is a
    fast non-casting HWDGE DMA (the SWDGE casting path delivered the first
    tile only after ~27us and delayed the whole K projection)
  - x_s/x_t are transposed by the DMA X-bar (dma_start transpose=True)
    straight from DRAM -> no PE transposes, no drain copies
  - PE order: k proj -> fire K AllGather in 2 chunks -> v proj -> fire V
    AllGather in 2 chunks (tt-major) -> q proj -> attention -> fuse.
    Collectives are issued from the otherwise-empty gpsimd queue.
  - gathered k lands in persistent kTf SBUF tiles (loads wait only on the
    collective), gathered v lands via strided DMA directly in the
    [V_h | 1] stationary layout (ones column -> softmax row sums for free)
  - attention in head pairs: st [128,2,512] f32 PSUM double-buffered so
    S^T(ki+1) overlaps exp(ki); the two S^T matmuls of a pair run
    concurrently in different PE row-tiles (tile_position)
  - deep pt pool so the exp stream can run ~32 iterations ahead of P@V
    while the V gather is still in flight
  - per-pair normalize: row sums staged at partition 0, DRAM-bounce
    partition broadcast, reciprocal_approx_fast, one multiply
"""

import sys

if "/opt/trn_rl_repo" not in sys.path:
    sys.path.insert(0, "/opt/trn_rl_repo")

import numpy as np

B, N, C, H, D = 2, 2048, 1024, 16, 64
NCORES = 8
T = (B * N) // NCORES          # 512 tokens per core
P = 128
SCALE = D ** -0.5              # 0.125
GROUPS = [[0, 1, 2, 3], [4, 5, 6, 7]]

# kt processing order: tt-major so P@V only needs v chunk tt after the vAG
# chunk covering that tt has landed. kt identifies (r=group member, tt).
KT_ORDER = [r * 4 + tt for tt in range(4) for r in range(4)]

_CACHE = {}


def _build():
    import concourse.bass as bass
    import concourse.mybir as mybir
    import concourse.tile as tile
    from concourse import bacc
    from concourse.masks import make_identity

    f32 = mybir.dt.float32
    bf16 = mybir.dt.bfloat16
    f8 = mybir.dt.float8e4      # e4m3: wire format for the k/v gathers

    nc = bacc.Bacc("TRN2", num_devices=NCORES, debug=False, enable_asserts=False)

    x_t = nc.dram_tensor("x_t", [T, C], bf16, kind="ExternalInput").ap()
    x_s = nc.dram_tensor("x_s", [T, C], bf16, kind="ExternalInput").ap()
    w_q = nc.dram_tensor("W_q", [C, C], bf16, kind="ExternalInput").ap()
    w_kv = nc.dram_tensor("W_kv", [C, 2 * C], bf16, kind="ExternalInput").ap()
    w_f = nc.dram_tensor("W_fuse", [C, C], bf16, kind="ExternalInput").ap()
    b_f = nc.dram_tensor("b_fuse", [1, C], f32, kind="ExternalInput").ap()
    out = nc.dram_tensor("out", [T, C], bf16, kind="ExternalOutput").ap()

    with tile.TileContext(nc) as tc:
        import contextlib

        with contextlib.ExitStack() as stk:
            consts = stk.enter_context(tc.tile_pool(name="consts", bufs=1))
            dram = stk.enter_context(tc.tile_pool(name="dram", bufs=1, space="DRAM"))

            identity = consts.tile([P, P], bf16, name="identity")
            make_identity(nc, identity)

            # preload the Exp activation table (saves 1.3us at first exp)
            dact = consts.tile([1, 2], f32, name="dact")
            nc.vector.memset(dact, 0.0)
            dact2 = consts.tile([1, 2], f32, name="dact2")
            nc.scalar.activation(dact2, dact,
                                 mybir.ActivationFunctionType.Exp, scale=1.0)

            bias_b = consts.tile([P, C], f32, name="bias_b")
            qT = [consts.tile([P, T], bf16, name=f"qT{m}") for m in range(8)]
            aT = [consts.tile([P, T], bf16, name=f"aT{c}") for c in range(8)]
            wf_b = consts.tile([P, 8, C], bf16, name="wf_b")
            wf = [wf_b[:, c, :] for c in range(8)]
            kTf = [consts.tile([P, 4, T], bf16, name=f"kTf{m}") for m in range(8)]
            vp = [consts.tile([P, H, D + 1], bf16, name=f"vp{kt}")
                  for kt in range(16)]

            # DRAM bounce buffers for the collectives. NOTE: fp8 wire format
            # was tried for k and/or v (halves collective bytes, -74us) but
            # pushed rel err to 2.05e-2/2.64e-2, over the 2e-2 gate.
            # K goes in ONE gather (all m-tiles ready together, early); V in
            # two dim-half gathers (heads 0-7 then 8-15) matching the pair
            # consumption order of the attention loop.
            k_in1 = dram.tile([4 * P * T], bf16, name="k_in1")   # m0-3
            k_in2 = dram.tile([4 * P * T], bf16, name="k_in2")   # m4-7
            k_out1 = dram.tile([16 * P * T], bf16, name="k_out1")
            k_out2 = dram.tile([16 * P * T], bf16, name="k_out2")
            v_in_lo = dram.tile([4 * P * 512], bf16, name="v_in_lo")
            v_in_hi = dram.tile([4 * P * 512], bf16, name="v_in_hi")
            v_out_lo = dram.tile([16 * P * 512], bf16, name="v_out_lo")
            v_out_hi = dram.tile([16 * P * 512], bf16, name="v_out_hi")
            rdram = dram.tile([H * T], f32, name="rdram")

            _k1r = k_in1.rearrange("(m p t) -> m p t", m=4, p=P, t=T)
            _k2r = k_in2.rearrange("(m p t) -> m p t", m=4, p=P, t=T)
            k_in_v = [_k1r[0], _k1r[1], _k1r[2], _k1r[3],
                      _k2r[0], _k2r[1], _k2r[2], _k2r[3]]
            v_in_lo_v = v_in_lo.rearrange("(q p c) -> q p c", q=4, p=P, c=512)
            v_in_hi_v = v_in_hi.rearrange("(q p c) -> q p c", q=4, p=P, c=512)
            _ko1 = k_out1.rearrange("(r m p t) -> m p r t", r=4, m=4, p=P, t=T)
            _ko2 = k_out2.rearrange("(r m p t) -> m p r t", r=4, m=4, p=P, t=T)
            k_out_v = [_ko1[0], _ko1[1], _ko1[2], _ko1[3],
                       _ko2[0], _ko2[1], _ko2[2], _ko2[3]]
            v_out_lo_v = v_out_lo.rearrange("(r q p c) -> r q p c",
                                            r=4, q=4, p=P, c=512)
            v_out_hi_v = v_out_hi.rearrange("(r q p c) -> r q p c",
                                            r=4, q=4, p=P, c=512)
            rdram_v = rdram.rearrange("(h t) -> h t", h=H)

            def cc_allgather(inb, outb):
                nc.gpsimd.collective_compute(
                    "AllGather", mybir.AluOpType.bypass, replica_groups=GROUPS,
                    ins=[inb[:].opt()], outs=[outb[:].opt()])

            # warm up the collective stream: the first CC op of a kernel pays
            # ~20-25us of startup; burn it on a 128B dummy while the k
            # projection is still running
            warm_in = dram.tile([128], bf16, name="warm_in")
            warm_out = dram.tile([512], bf16, name="warm_out")
            cc_allgather(warm_in, warm_out)

            # ---------------- phase A: projections ----------------
            # pak outlives phase A: wk/xsT/kl feed the m4-7 K projection
            # that is interleaved into attention pair 0
            pak = stk.enter_context(tc.tile_pool(name="pak", bufs=1))
            with tc.tile_pool(name="pa", bufs=1) as pa, \
                 tc.tile_pool(name="kp_ps", bufs=1, space="PSUM") as kp_ps, \
                 tc.tile_pool(name="tp_ps", bufs=2, space="PSUM") as tp_ps, \
                 tc.tile_pool(name="pp_ps", bufs=2, space="PSUM") as pp_ps:

                # batched loads: few big 3D-AP DMAs instead of many small
                # tiles — per-DMA latency (~3-6us each) was pacing phase A.
                # x on the sync queue, weights on the gpsimd queue.
                xs_b = pa.tile([P, 4, C], bf16, name="xs_b")
                xs_r = x_s.rearrange("(i p) c -> p i c", p=P)
                xt_b = pa.tile([P, 4, C], bf16, name="xt_b")
                xt_r = x_t.rearrange("(i p) c -> p i c", p=P)
                # first x_s row-tile alone so the transposes start ~6us sooner
                nc.sync.dma_start(out=xs_b[:, 0:1, :], in_=xs_r[:, 0:1, :])
                nc.sync.dma_start(out=xs_b[:, 1:4, :], in_=xs_r[:, 1:4, :])
                for h in range(2):
                    nc.sync.dma_start(out=xt_b[:, 2 * h:2 * h + 2, :],
                                      in_=xt_r[:, 2 * h:2 * h + 2, :])
                xs_nat = [xs_b[:, i, :] for i in range(4)]
                xt_nat = [xt_b[:, i, :] for i in range(4)]

                wk_b = pak.tile([P, 8, C], bf16, name="wk_b")
                wk_r = w_kv[:, 0:C].rearrange("(c p) k -> p c k", p=P)
                wv_b = pa.tile([P, 8, C], bf16, name="wv_b")
                wv_r = w_kv[:, C:2 * C].rearrange("(c p) k -> p c k", p=P)
                wq_b = pa.tile([P, 8, C], bf16, name="wq_b")
                wq_r = w_q.rearrange("(c p) k -> p c k", p=P)
                for h in range(2):
                    nc.gpsimd.dma_start(out=wk_b[:, 4 * h:4 * h + 4, :],
                                        in_=wk_r[:, 4 * h:4 * h + 4, :])
                for h in range(2):
                    nc.gpsimd.dma_start(out=wv_b[:, 4 * h:4 * h + 4, :],
                                        in_=wv_r[:, 4 * h:4 * h + 4, :])
                wk = [wk_b[:, c, :] for c in range(8)]
                wv = [wv_b[:, c, :] for c in range(8)]
                wq = [wq_b[:, c, :] for c in range(8)]

                def transpose_in(nat, dstT):
                    for i in range(4):
                        for c in range(8):
                            pst = tp_ps.tile([P, P], bf16, name="pst")
                            nc.tensor.transpose(
                                pst, nat[i][:, c * P:(c + 1) * P], identity)
                            nc.vector.tensor_copy(
                                out=dstT[c][:, i * P:(i + 1) * P], in_=pst)

                xsT = [pak.tile([P, T], bf16, name=f"xsT{c}") for c in range(8)]
                transpose_in(xs_nat, xsT)

                # k^T projection m0-3 only (pairs 0-3); m4-7 is deferred and
                # interleaved into attention pair 0, where PE would otherwise
                # idle waiting for the V gather
                pss = [kp_ps.tile([P, T], f32, name="kps", tag=f"kps{j}")
                       for j in range(4)]
                for c in range(8):
                    for j in range(4):
                        nc.tensor.matmul(
                            pss[j], wk[c][:, j * P:(j + 1) * P], xsT[c],
                            start=(c == 0), stop=(c == 7))
                for j in range(4):
                    kl = pak.tile([P, T], bf16, name="kl", bufs=4)
                    nc.vector.tensor_copy(out=kl, in_=pss[j])
                    nc.gpsimd.dma_start(out=k_in_v[j], in_=kl)
                cc_allgather(k_in1, k_out1)

                # wq rides the gpsimd queue behind the k_in writes; vp ones
                # columns prime on vector once the kl drains are queued
                for h in range(2):
                    nc.gpsimd.dma_start(out=wq_b[:, 4 * h:4 * h + 4, :],
                                        in_=wq_r[:, 4 * h:4 * h + 4, :])
                for kt in range(16):
                    nc.vector.memset(vp[kt], 1.0)

                # v projection (natural layout), nh-outer: the lo dim-half
                # (heads 0-7, consumed by pairs 0-3) completes and gathers
                # first; kAG2 slots between the two v gathers (its consumers
                # run last)
                for nh in range(2):
                    for tt in range(4):
                        ps = pp_ps.tile([P, 512], f32, name="proj_ps")
                        for c in range(8):
                            nc.tensor.matmul(
                                ps,
                                xsT[c][:, tt * P:(tt + 1) * P],
                                wv[c][:, nh * 512:(nh + 1) * 512],
                                start=(c == 0), stop=(c == 7))
                        vl = pa.tile([P, 512], bf16, name="vl", bufs=4)
                        nc.vector.tensor_copy(out=vl, in_=ps)
                        nc.gpsimd.dma_start(
                            out=[v_in_lo_v, v_in_hi_v][nh][tt], in_=vl)
                    if nh == 0:
                        cc_allgather(v_in_lo, v_out_lo)
                # kAG2 and vAG-hi are emitted from inside attention pair 0,
                # once the interleaved m4-7 projection lands

                # q^T projection
                xtT = [pa.tile([P, T], bf16, name=f"xtT{c}") for c in range(8)]
                transpose_in(xt_nat, xtT)
                for m in range(8):
                    ps = pp_ps.tile([P, T], f32, name="proj_ps")
                    for c in range(8):
                        nc.tensor.matmul(ps, wq[c][:, m * P:(m + 1) * P], xtT[c],
                                         start=(c == 0), stop=(c == 7))
                    nc.vector.tensor_copy(out=qT[m], in_=ps)

                # late loads: W_fuse + bias ride behind the collectives
                wf_r = w_f.rearrange("(c p) k -> p c k", p=P)
                for h in range(2):
                    nc.gpsimd.dma_start(out=wf_b[:, 4 * h:4 * h + 4, :],
                                        in_=wf_r[:, 4 * h:4 * h + 4, :])
                nc.gpsimd.dma_start(out=bias_b, in_=b_f.to_broadcast([P, C]))

            # gathered k m0-3 -> kTf (sync queue, dep kAG1/kAG1b); the rest on
            # the gpsimd queue in collective-completion order: vp-lo (vAG-lo),
            # kTf m4-7 (kAG2), vp-hi (vAG-hi)
            for m in range(4):
                nc.sync.dma_start(out=kTf[m], in_=k_out_v[m])
            for kt in KT_ORDER:
                r, ttv = kt // 4, kt % 4
                nc.gpsimd.dma_start(
                    out=vp[kt][:, 0:8, 0:D],
                    in_=v_out_lo_v[r, ttv].rearrange("p (h d) -> p h d", h=8))
            # (kTf m4-7 and vp-hi loads are emitted after attention pair 0 —
            # they must follow the kAG2/vAG-hi emission in program order for
            # dependency tracking, and must not sit on the gpsimd queue ahead
            # of those launches)

            # ---------------- phase B: attention ----------------
            with tc.tile_pool(name="ptp", bufs=30) as ptp, \
                 tc.tile_pool(name="sm", bufs=2) as sm:

                def emit_st(stp, hp, ki):
                    kt = KT_ORDER[ki]
                    r, tcol = kt // 4, (kt % 4) * P
                    st = stp.tile([P, 2, T], f32, name="st")
                    for sub in range(2):
                        nc.tensor.matmul(
                            st[:, sub, :],
                            kTf[hp][sub * D:(sub + 1) * D, r, tcol:tcol + P],
                            qT[hp][sub * D:(sub + 1) * D, :],
                            start=True, stop=True,
                            tile_position=(sub * D, 0))
                    return st

                def pair_body(hp, stp, otp, extra):
                    ot = [otp.tile([D + 1, T], f32, name="ot", tag=f"ot{i}")
                          for i in range(2)]
                    st = emit_st(stp, hp, 0)
                    for ki in range(16):
                        kt = KT_ORDER[ki]
                        pt = ptp.tile([P, 2, T], bf16, name="pt")
                        nc.scalar.activation(
                            pt[:], st[:],
                            mybir.ActivationFunctionType.Exp, scale=SCALE)
                        if ki < 15:
                            st = emit_st(stp, hp, ki + 1)
                        for i in range(2):
                            nc.tensor.matmul(
                                ot[i], vp[kt][:, 2 * hp + i, :], pt[:, i, :],
                                start=(ki == 0), stop=(ki == 15))
                        if extra is not None:
                            extra(2 * ki)
                            extra(2 * ki + 1)
                    # drain pair: O^T -> aT[hp]; row sums staged at partition
                    # 0, DRAM-bounced into a partition broadcast, then one
                    # fast reciprocal + multiply normalizes the pair
                    for i in range(2):
                        nc.vector.tensor_copy(
                            out=aT[hp][i * D:(i + 1) * D, :], in_=ot[i][0:D, :])
                        rs = sm.tile([1, T], f32, name="rs", tag=f"rs{i}")
                        nc.vector.tensor_copy(out=rs, in_=ot[i][D:D + 1, :])
                        nc.sync.dma_start(out=rdram_v[2 * hp + i], in_=rs)
                    rb = sm.tile([P, T], f32, name="rb")
                    for half in range(2):
                        bcast = bass.AP(
                            tensor=rdram.tensor,
                            offset=rdram.offset + (2 * hp + half) * T,
                            ap=[[0, D], [1, T]])
                        nc.gpsimd.dma_start(
                            out=rb[half * D:(half + 1) * D, :], in_=bcast)
                    rbi = sm.tile([P, T], f32, name="rbi")
                    nc.vector.reciprocal_approx_fast(out=rbi, in_=rb)
                    nc.vector.tensor_mul(out=aT[hp], in0=aT[hp], in1=rbi)

                # the deferred m4-7 K projection runs as a PREFIX of pair 0 —
                # exactly the window where PE otherwise idles waiting for the
                # kAG1 results; kAG2 + vAG-hi fire ~50us earlier than when
                # this work was interleaved behind the first S^T
                with tc.tile_pool(name="st0_ps", bufs=2, space="PSUM") as st0, \
                     tc.tile_pool(name="ot0_ps", bufs=1, space="PSUM") as ot0, \
                     tc.tile_pool(name="kp2_ps", bufs=1, space="PSUM") as kp2:
                    for sub in range(2):
                        pss2 = [kp2.tile([P, T], f32, name="kps2",
                                         tag=f"kps2_{j}") for j in range(2)]
                        for c in range(8):
                            for j in range(2):
                                m = 4 + 2 * sub + j
                                nc.tensor.matmul(
                                    pss2[j], wk[c][:, m * P:(m + 1) * P],
                                    xsT[c], start=(c == 0), stop=(c == 7))
                        for j in range(2):
                            m = 4 + 2 * sub + j
                            kl = pak.tile([P, T], bf16, name="kl", bufs=4)
                            nc.vector.tensor_copy(out=kl, in_=pss2[j])
                            nc.gpsimd.dma_start(out=k_in_v[m], in_=kl)
                    cc_allgather(k_in2, k_out2)
                    cc_allgather(v_in_hi, v_out_hi)

                    pair_body(0, st0, ot0, None)

                # now that kAG2/vAG-hi exist in program order, load their
                # results (sync queue; gpsimd carries the launches)
                for m in range(4, 8):
                    nc.sync.dma_start(out=kTf[m], in_=k_out_v[m])
                for kt in KT_ORDER:
                    r, ttv = kt // 4, kt % 4
                    nc.sync.dma_start(
                        out=vp[kt][:, 8:16, 0:D],
                        in_=v_out_hi_v[r, ttv].rearrange(
                            "p (h d) -> p h d", h=8))

                with tc.tile_pool(name="st_ps", bufs=2, space="PSUM") as st_ps, \
                     tc.tile_pool(name="ot_ps", bufs=2, space="PSUM") as ot_ps:
                    for hp in range(1, 8):
                        pair_body(hp, st_ps, ot_ps, None)

            # ---------------- phase C: fuse projection ----------------
            with tc.tile_pool(name="fu", bufs=4) as fu, \
                 tc.tile_pool(name="fu_ps", bufs=4, space="PSUM") as fu_ps:
                for tt in range(4):
                    for nh in range(2):
                        ps = fu_ps.tile([P, 512], f32, name="fps")
                        for c in range(8):
                            nc.tensor.matmul(
                                ps, aT[c][:, tt * P:(tt + 1) * P],
                                wf[c][:, nh * 512:(nh + 1) * 512],
                                start=(c == 0), stop=(c == 7))
                        ob = fu.tile([P, 512], bf16, name="ob")
                        nc.vector.tensor_add(
                            out=ob, in0=ps, in1=bias_b[:, nh * 512:(nh + 1) * 512])
                        nc.sync.dma_start(
                            out=out[tt * P:(tt + 1) * P, nh * 512:(nh + 1) * 512],
                            in_=ob)

    nc.compile()
    return nc


def _get_nc():
    if "nc" not in _CACHE:
        _CACHE["nc"] = _build()
    return _CACHE["nc"]


def make_in_maps(inputs):
    """Shard + host-cast the full inputs into per-core input maps."""
    import ml_dtypes

    bf16 = ml_dtypes.bfloat16
    x_t = np.asarray(inputs["x_t"]).reshape(B * N, C).astype(bf16)
    x_s = np.asarray(inputs["x_s"]).reshape(B * N, C).astype(bf16)
    w_q = np.asarray(inputs["W_q"]).astype(bf16)
    w_kv = np.asarray(inputs["W_kv"]).astype(bf16)
    w_f = np.asarray(inputs["W_fuse"]).astype(bf16)
    b_f = np.asarray(inputs["b_fuse"]).astype(np.float32).reshape(1, C)

    in_maps = []
    for i in range(NCORES):
        in_maps.append({
            "x_t": x_t[i * T:(i + 1) * T],
            "x_s": x_s[i * T:(i + 1) * T],
            "W_q": w_q,
            "W_kv": w_kv,
            "W_fuse": w_f,
            "b_fuse": b_f,
        })
    return in_maps


def kernel(**inputs):
    nc = _get_nc()
    from concourse import bass_utils

    in_maps = make_in_maps(inputs)
    res = bass_utils.run_bass_kernel_spmd(nc, in_maps, core_ids=list(range(NCORES)))
    out = np.concatenate([res.results[i]["out"] for i in range(NCORES)], axis=0)
    return out.reshape(B, N, C).astype(np.float32)


if __name__ == "__main__":
    _build()
    print("build+compile OK")
